# revision 50
# baseline (speedup 1.0000x reference)
"""Trainium2 Bass kernel for KL-divergence attention.

Math used (exactly equivalent to the reference model):
  q = x@Wq, k = x@Wk, v = x@Wv
  kl_ij  = sum_h p_i log p_i - p_i . logq_j   (p = softmax(q), logq = log_softmax(k))
  attn   = softmax_j(-kl_ij) = softmax_j(p_i . logq_j)     [neg-entropy cancels]
         = softmax_j(p_i . k_j - lse_j)
  With exp(s - lse_j) = exp(s)/sk_j (sk_j = sum_h exp(k_jh)), the 1/sk_j
  factor is absorbed into the V rows and the softmax-denominator matmul, so
  no log is needed. With e_ij = exp(p_i . k_j), v'_j = 256 v_j / sk_j,
  r'_j = 256 / sk_j:
    out_i = (sum_j e_ij v'_j) / (sum_j e_ij r'_j)

Precision strategy (validated against a numpy simulation of the full cast
chain): the attention output is a near-cancellation (diffuse weights
averaging zero-mean v rows), so per-element quantization noise on e or v
passes straight through to the output — naive fp8 there costs ~2.6e-2 rel
err. Instead the output GEMM is CENTERED: e = 1 + f, so

    num_i = colsum(v') + sum_j f_ij v'_j
    den_i = colsum(r') + sum_j f_ij r'_j

The rank-1 colsum terms are computed exactly from fp16 v' during phase 2
and injected into each PSUM chain with one K=1 matmul; only the small
residual f (|f| ~ 0.3, quantization noise ~5x below e's) is contracted in
fp8 DoubleRow. Full error budget (hardware): ~1.2e-2 vs the 2e-2 gate.

Per-GEMM precisions:
  - q/k projections: fp8e4 DoubleRow (x fp8; W pre-scaled x16 into fp8's
    normal range; q_ps = 16q, undone by the exp's scale)
  - kT: computed DIRECTLY as a second DoubleRow projection (Wk stationary,
    xT moving -> [h, j] layout), replacing per-block PE transposes + casts
  - pairwise scores GEMM: fp8 DoubleRow (pT = 128p, kT = 16k, exp scale
    1/2048; p*128 <= 128 < 240 so no fp8 overflow is possible)
  - v projection: fp16 (v feeds the output linearly — fp8 unacceptable)
  - f residual + output GEMM + denominator: fp8 DoubleRow over the
    centered residual; exp -> fp16 on Scalar, the -1 subtract + fp8 cast
    runs on the otherwise-idle GpSimd engine (SBUF->SBUF)
  - denominator contraction: REVERSED matmuls (lhsT = r' column pair ->
    2-column LDWEIGHTS, ~free) into a [1, 512] PSUM row per i-group,
    transposed back onto partitions with 4 tiny fp32 identity matmuls
  - scores are computed TRANSPOSED (sT[j,i] = kT.T @ pT) so f feeds the
    output GEMM as its stationary operand with no transpose
  - the p transpose runs against diag(128/sq) (fp16), folding the softmax
    normalization and fp8 pre-scale into the transpose matmul
"""

import numpy as np

import concourse.bass as bass
import concourse.tile as tile
from concourse import bacc, mybir
from concourse.bass_utils import run_bass_kernel_spmd
from concourse.masks import make_identity

B, S, D, H = 32, 2048, 512, 512
NCORES = 8
BPC = B // NCORES  # batches per core
P = 128
NB = S // P   # 16 row blocks per batch
ND = D // P   # 4 contraction chunks
NH = H // P   # 4 h chunks
NG = 4        # i groups in phase 3
GW = S // NG  # 512 i columns per group
NPAIR = NB // 2

FP32 = mybir.dt.float32
FP16 = mybir.dt.float16
FP8 = mybir.dt.float8e4
EXP = mybir.ActivationFunctionType.Exp
DR = mybir.MatmulPerfMode.DoubleRow
MUL = mybir.AluOpType.mult
SUB = mybir.AluOpType.subtract

WS = 16.0        # host-side Wq/Wk pre-scale (keeps fp8 W in normal range)
PS = 128.0       # p pre-scale: p*PS <= 128 < 240 (fp8e4 max) always safe
SK_SCALE = 256.0  # v/rsk pre-scale; cancels between numerator/denominator


def _emit(tc):
    # Inputs arrive pre-sharded/pre-laid-out by the host side of kernel():
    # x as [BPC, D, S] in BOTH fp8 (q/k path) and fp16 (v path), each W as
    # [128, ND, H] chunked on the contraction dim (Wq/Wk fp8 pre-scaled by
    # WS, Wv fp16 unscaled).
    nc = tc.nc
    x8 = nc.dram_tensor("x8", [BPC, D, S], FP8, kind="ExternalInput").ap()
    x16 = nc.dram_tensor("x16", [BPC, D, S], FP16, kind="ExternalInput").ap()
    wq = nc.dram_tensor("Wq", [P, ND, H], FP8, kind="ExternalInput").ap()
    wk = nc.dram_tensor("Wk", [P, ND, H], FP8, kind="ExternalInput").ap()
    wv = nc.dram_tensor("Wv", [P, ND, H], FP16, kind="ExternalInput").ap()
    out = nc.dram_tensor("out", [BPC, S, H], FP32, kind="ExternalOutput").ap()

    import contextlib

    with contextlib.ExitStack() as ctx:
        consts = ctx.enter_context(tc.tile_pool(name="consts", bufs=1))
        big = ctx.enter_context(tc.tile_pool(name="big", bufs=1))
        vpool = ctx.enter_context(tc.tile_pool(name="vpool", bufs=2))
        fpool = ctx.enter_context(tc.tile_pool(name="fpool", bufs=2))
        stage = ctx.enter_context(tc.tile_pool(name="stage", bufs=4))
        small = ctx.enter_context(tc.tile_pool(name="small", bufs=4))
        dpool = ctx.enter_context(tc.tile_pool(name="dpool", bufs=2))
        outp = ctx.enter_context(tc.tile_pool(name="outp", bufs=4))
        psp = ctx.enter_context(tc.tile_pool(name="psp", bufs=4, space="PSUM"))

        ident32 = consts.tile([P, P], FP32)
        make_identity(nc, ident32)
        ident16 = consts.tile([P, P], FP16)
        nc.vector.tensor_copy(ident16, ident32)
        ones32 = consts.tile([1, 1], FP32)
        nc.vector.memset(ones32, 1.0)
        ones32c = consts.tile([P, 1], FP32)
        nc.vector.memset(ones32c, 1.0)
        ones16c = consts.tile([P, 1], FP16)
        nc.vector.memset(ones16c, 1.0)
        ones16r = consts.tile([1, GW], FP16)
        nc.vector.memset(ones16r, 1.0)

        # Weights arrive pre-chunked/pre-scaled; straight DMA.
        # wk first on the Sync ring (the opening kT-direct chains need only
        # wk + x8-q0); wq/wv go out on the GpSimd ring in parallel (first
        # needed by the projections, several microseconds in)
        wk_f = consts.tile([P, ND, H], FP8, name="wk_sb")
        nc.sync.dma_start(out=wk_f, in_=wk)
        wq_f = consts.tile([P, ND, H], FP8, name="wq_sb")
        nc.sync.dma_start(out=wq_f, in_=wq)
        wv_f = consts.tile([P, ND, H], FP16, name="wv_sb")

        def alloc_and_dma_x(b):
            # ---- phase 1: xT arrives pre-transposed — straight DMA ----
            xT8 = big.tile([P, ND, S], FP8, tag="xT8", bufs=2, name=f"xT8_{b}")
            xT16 = big.tile([P, ND, S], FP16, tag="xT16", bufs=2,
                            name=f"xT16_{b}")
            xsrc8 = x8[b].rearrange("(c p) s -> p c s", p=P)
            xsrc16 = x16[b].rearrange("(c p) s -> p c s", p=P)
            # x8 lands first (kT-direct + q/k projections need it); x16
            # (v path) follows
            for q4 in range(4):
                cols = slice(q4 * GW, (q4 + 1) * GW)
                nc.sync.dma_start(out=xT8[:, :, cols], in_=xsrc8[:, :, cols])
                if q4 == 0:
                    nc.sync.dma_start(out=xT16[:, :, cols],
                                      in_=xsrc16[:, :, cols])
            for q4 in range(1, 4):
                cols = slice(q4 * GW, (q4 + 1) * GW)
                nc.sync.dma_start(out=xT16[:, :, cols], in_=xsrc16[:, :, cols])
            kT = big.tile([P, NH, S], FP8, tag="kT", bufs=2, name=f"kT_{b}")
            return {"xT8": xT8, "xT16": xT16, "kT": kT}

        def emit_ktd_chain(tiles, jq, hc):
            # kT computed directly: kT[h, j] = sum_d Wk[d, h] xT[d, j]
            jcols = slice(jq * GW, (jq + 1) * GW)
            kt_ps = psp.tile([P, GW], FP32, tag="ps", name="kt_ps")
            for c in range(ND // 2):
                nc.tensor.matmul(
                    kt_ps,
                    wk_f[:, 2 * c:2 * c + 2, hc * P:(hc + 1) * P],
                    tiles["xT8"][:, 2 * c:2 * c + 2, jcols],
                    start=(c == 0), stop=(c == ND // 2 - 1),
                    perf_mode=DR)
            nc.vector.tensor_copy(out=tiles["kT"][:, hc, jcols], in_=kt_ps)

        cur = None
        pending = None
        for b in range(BPC):
            if cur is None:
                cur = alloc_and_dma_x(0)
                nc.sync.dma_start(out=wv_f, in_=wv)
                for jq in range(4):
                    for hc in range(NH):
                        emit_ktd_chain(cur, jq, hc)
            xT8, xT16, kT = cur["xT8"], cur["xT16"], cur["kT"]

            # ---- phase 2: projections, softmax pieces, transposed p ----
            pT = big.tile([P, NH, S], FP8, tag="pT", bufs=2, name=f"pT_{b}")
            # v rows scaled by 256/sk: fp8 copy for the f-GEMM, fp16
            # transients for the exact colsum
            v8_all = vpool.tile([P, NB, H], FP8, tag="v8", name=f"v8_{b}")
            rsk16_all = small.tile([P, NB], FP16, tag="rsk16", bufs=2)
            r8_all = small.tile([P, NB, 16], FP8, tag="r8", bufs=2)
            cs_ps = psp.tile([1, H], FP32, tag="ps2", bufs=2, name="cs_ps")
            eq_tiles = {}
            diag_tiles = {}

            def emit_tr(jb):
                tp = psp.tile([P, H], FP32, tag="ps", name="tpp")
                for hc in range(NH):
                    nc.tensor.matmul(
                        tp[:, hc * P:(hc + 1) * P],
                        eq_tiles[jb][:, hc * P:(hc + 1) * P], diag_tiles[jb],
                        start=True, stop=True)
                nc.vector.tensor_copy(
                    out=pT[:, :, jb * P:(jb + 1) * P],
                    in_=tp.rearrange("p (c f) -> p c f", c=NH))

            for ib in range(NB):
                q_ps = psp.tile([P, H], FP32, tag="ps", name="q_ps")
                k_ps = psp.tile([P, H], FP32, tag="ps", name="k_ps")
                for ps, wt in ((q_ps, wq_f), (k_ps, wk_f)):
                    for c in range(ND // 2):
                        nc.tensor.matmul(
                            ps, xT8[:, 2 * c:2 * c + 2, ib * P:(ib + 1) * P],
                            wt[:, 2 * c:2 * c + 2, :],
                            start=(c == 0), stop=(c == ND // 2 - 1),
                            perf_mode=DR)
                v_ps = psp.tile([P, H], FP32, tag="ps", name="v_ps")
                for dc in range(ND):
                    nc.tensor.matmul(
                        v_ps, xT16[:, dc, ib * P:(ib + 1) * P], wv_f[:, dc, :],
                        start=(dc == 0), stop=(dc == ND - 1))

                eq_sb = stage.tile([P, H], FP16, tag="eq", bufs=5)
                sq = small.tile([P, 1], FP32, tag="sq")
                nc.scalar.activation(eq_sb, q_ps, EXP, scale=1.0 / WS,
                                     accum_out=sq)
                rq = small.tile([P, 1], FP32, tag="rq")
                nc.vector.reciprocal(rq, sq)
                # diag(PS/sq): folds p-normalization and the fp8 pre-scale
                # into the p transpose matmul
                diag = stage.tile([P, P], FP16, tag="diag", bufs=5)
                nc.gpsimd.tensor_scalar(diag, ident16, rq, PS, op0=MUL,
                                        op1=MUL)

                ek_sb = stage.tile([P, H], FP16, tag="ek", bufs=2)
                sk = small.tile([P, 1], FP32, tag="sk")
                nc.scalar.activation(ek_sb, k_ps, EXP, scale=1.0 / WS,
                                     accum_out=sk)
                rsk = small.tile([P, 1], FP32, tag="rsk")
                nc.vector.reciprocal(rsk, sk)
                nc.vector.tensor_scalar_mul(
                    rsk16_all[:, ib:ib + 1], rsk, SK_SCALE)
                nc.vector.tensor_scalar_mul(
                    r8_all[:, ib, 0:1], rsk, SK_SCALE)
                # v' = v * rsk * SK_SCALE: fp16 transient for the exact
                # colsum, fp8 copy (via GpSimd) for the f-GEMM
                v16_sb = stage.tile([P, H], FP16, tag="v16", bufs=3)
                nc.vector.tensor_scalar(
                    v16_sb, v_ps, rsk, SK_SCALE, op0=MUL, op1=MUL)
                nc.gpsimd.tensor_copy(out=v8_all[:, ib, :], in_=v16_sb)
                nc.tensor.matmul(cs_ps, ones16c, v16_sb,
                                 start=(ib == 0), stop=(ib == NB - 1))

                eq_tiles[ib] = eq_sb
                diag_tiles[ib] = diag
                if ib >= 3:
                    emit_tr(ib - 3)

            # exact colsum(r'): free-dim reduce on DVE, then one tiny matmul
            # colsum finalization is deferred into the scores stream (see
            # below) so the phase-2-end Vector backlog never stalls the PE

            # ---- phase 3: scores (transposed, DR), f = e - 1, output ----
            def emit_scores(gp, jbs, fb):
                for jb in jbs:
                    s_ps = psp.tile([P, 2, GW], FP32, tag="ps2", bufs=2,
                                    name="s_ps")
                    for pair in range(2):
                        lhs = kT[:, 2 * pair:2 * pair + 2,
                                 jb * P:(jb + 1) * P]
                        for lg in range(2):
                            ig = 2 * gp + lg
                            nc.tensor.matmul(
                                s_ps[:, lg, :], lhs,
                                pT[:, 2 * pair:2 * pair + 2,
                                   ig * GW:(ig + 1) * GW],
                                start=(pair == 0), stop=(pair == 1),
                                perf_mode=DR)
                    e16 = stage.tile([P, 2 * GW], FP16, tag="e16", bufs=4)
                    nc.scalar.activation(
                        e16, s_ps.rearrange("p a f -> p (a f)"), EXP,
                        scale=1.0 / (PS * WS))
                    nc.vector.tensor_scalar(
                        fb[:, jb, :], e16, 1.0, None, op0=SUB)

            def emit_den(gp, fb):
                # denominators: colsum(r') broadcast + reversed DR matmuls
                # over the f residual, then the [1, 512] PSUM row is
                # transposed back onto partitions with 4 tiny fp32 matmuls.
                # both den chains first, then the rl transposes — the
                # PSUM->SBUF copy of den(lg0) hides under den(lg1)'s chain
                den_sbs = {}
                for lg in range(2):
                    den_ps = psp.tile([1, GW], FP32, tag="ps", name="den_ps")
                    nc.tensor.matmul(den_ps, csr_sb, ones16r,
                                     start=True, stop=False)
                    for t in range(NPAIR):
                        nc.tensor.matmul(
                            den_ps, r8_all[:, 2 * t:2 * t + 2, 0:1],
                            fb[:, 2 * t:2 * t + 2, lg * GW:(lg + 1) * GW],
                            start=False, stop=(t == NPAIR - 1),
                            perf_mode=DR)
                    den_sb = dpool.tile([1, GW], FP32, tag="den")
                    nc.vector.tensor_copy(den_sb, den_ps)
                    den_sbs[lg] = den_sb
                rl_sbs = {}
                for lg in range(2):
                    rl_ps = psp.tile([P, NG], FP32, tag="ps", name="rl_ps")
                    for il in range(NG):
                        nc.tensor.matmul(
                            rl_ps[:, il:il + 1],
                            den_sbs[lg][0:1, il * P:(il + 1) * P], ones32,
                            start=True, stop=True)
                    rl_sb = small.tile([P, NG], FP32, tag="rl", bufs=4)
                    nc.vector.reciprocal(rl_sb, rl_ps)
                    rl_sbs[lg] = rl_sb
                return rl_sbs

            def emit_out_chain(st, lg, il):
                ig = 2 * st["gp"] + lg
                ib = ig * NG + il
                o_ps = psp.tile([P, H], FP32, tag="ps", name="o_ps")
                nc.tensor.matmul(o_ps, ones16r[0:1, 0:P], st["cs"],
                                 start=True, stop=False)
                for t in range(NPAIR):
                    nc.tensor.matmul(
                        o_ps,
                        st["fb"][:, 2 * t:2 * t + 2,
                                 lg * GW + il * P:lg * GW + (il + 1) * P],
                        st["v8"][:, 2 * t:2 * t + 2, :],
                        start=False, stop=(t == NPAIR - 1),
                        perf_mode=DR)
                o_sb = outp.tile([P, H], FP32, tag="o")
                nc.vector.tensor_scalar_mul(
                    o_sb, o_ps, st["rl"][lg][:, il:il + 1])
                # output stores go out on the GpSimd queue so the next
                # batch's input loads (Sync queue) never queue behind them
                nc.gpsimd.dma_start(
                    out=out[st["b"], ib * P:(ib + 1) * P, :], in_=o_sb)

            # The scores loops are Scalar-bound (the PE needs ~0.9us per
            # row block but the exp takes ~1.2us), so PE-pure work is
            # interleaved into them to fill the bubbles:
            #   group 0: the NEXT batch's kT-direct chains (its x DMAs are
            #            prefetched here, overlapping this batch's compute)
            #            plus the PREVIOUS batch's deferred group-1 output
            #   group 1: group-0's output chains
            # The first 13 gp0 blocks are hoisted in front of the tail p
            # transposes so the latter never stall the PE on the exp
            # backlog; den(0) trails two gp1 blocks to hide the wait on
            # gp0's last f-subtract.
            nxt = alloc_and_dma_x(b + 1) if b + 1 < BPC else None
            ktd_todo = [(jq, hc) for jq in range(4) for hc in range(NH)] \
                if nxt is not None else []
            out1_todo = [(pending, lg, il) for lg in range(2)
                         for il in range(NG)] if pending is not None else []
            fb0 = fpool.tile([P, NB, 2 * GW], FP8, tag="f", name=f"f0_{b}")
            for jb in range(NB - 3):
                emit_scores(0, [jb], fb0)
                if ktd_todo:
                    emit_ktd_chain(nxt, *ktd_todo.pop(0))
                if jb % 2 == 0 and out1_todo:
                    emit_out_chain(*out1_todo.pop(0))
                if jb == 0:
                    # deferred colsum(r') finalization: free-dim reduce on
                    # DVE, then one tiny matmul for the cross-partition sum
                    rsum = small.tile([P, 1], FP32, tag="rsum", bufs=2)
                    nc.vector.tensor_reduce(rsum, rsk16_all,
                                            mybir.AxisListType.X,
                                            mybir.AluOpType.add)
                    csr_ps = psp.tile([1, 1], FP32, tag="ps", name="csr_ps")
                    nc.tensor.matmul(csr_ps, rsum, ones32c,
                                     start=True, stop=True)
                    csr_sb = small.tile([1, 1], FP16, tag="csr", bufs=2)
                    nc.vector.tensor_copy(csr_sb, csr_ps)
                elif jb == 1:
                    cs_sb = dpool.tile([1, H], FP16, tag="cs")
                    nc.vector.tensor_copy(cs_sb, cs_ps)
            emit_tr(NB - 3)
            emit_tr(NB - 2)
            emit_tr(NB - 1)
            for jb in range(NB - 3, NB):
                emit_scores(0, [jb], fb0)
                if ktd_todo:
                    emit_ktd_chain(nxt, *ktd_todo.pop(0))
                if out1_todo:
                    emit_out_chain(*out1_todo.pop(0))
            fb1 = fpool.tile([P, NB, 2 * GW], FP8, tag="f", name=f"f1_{b}")
            emit_scores(1, [0], fb1)
            while ktd_todo:
                emit_ktd_chain(nxt, *ktd_todo.pop(0))
            while out1_todo:
                emit_out_chain(*out1_todo.pop(0))
            emit_scores(1, [1], fb1)
            rl0 = emit_den(0, fb0)
            st0 = {"b": b, "gp": 0, "fb": fb0, "v8": v8_all, "rl": rl0,
                   "cs": cs_sb}
            for jb in range(2, NB):
                emit_scores(1, [jb], fb1)
                if jb % 2 == 1:
                    c = jb // 2
                    emit_out_chain(st0, c // NG, c % NG)
            emit_out_chain(st0, 0, 0)
            rl1 = emit_den(1, fb1)
            pending = {"b": b, "gp": 1, "fb": fb1, "v8": v8_all, "rl": rl1,
                       "cs": cs_sb}
            cur = nxt

        # drain the last batch's deferred group-1 output
        for lg in range(2):
            for il in range(NG):
                emit_out_chain(pending, lg, il)


_NC_CACHE = {}


def _get_nc():
    if "nc" not in _NC_CACHE:
        nc = bacc.Bacc("TRN2", target_bir_lowering=False, debug=False)
        with tile.TileContext(nc) as tc:
            _emit(tc)
        nc.compile()
        _NC_CACHE["nc"] = nc
    return _NC_CACHE["nc"]


_NP8 = mybir.dt.np(FP8)


def _prep_w(w, scale, npdt):
    # [D, H] fp32 -> [128, ND, H] chunked on the contraction dim, *scale
    w = np.asarray(w, dtype=np.float32).reshape(ND, P, H).transpose(1, 0, 2)
    return np.ascontiguousarray(w * scale).astype(npdt)


def _run(inputs, trace=False, trace_cores=None):
    nc = _get_nc()
    x = np.asarray(inputs["x"], dtype=np.float32)
    wq = _prep_w(inputs["Wq"], WS, _NP8)
    wk = _prep_w(inputs["Wk"], WS, _NP8)
    wv = _prep_w(inputs["Wv"], 1.0, np.float16)
    in_maps = []
    for c in range(NCORES):
        xs = x[c * BPC:(c + 1) * BPC]
        xt = np.ascontiguousarray(xs.transpose(0, 2, 1))
        in_maps.append({
            "x8": xt.astype(_NP8), "x16": xt.astype(np.float16),
            "Wq": wq, "Wk": wk, "Wv": wv,
        })
    res = run_bass_kernel_spmd(
        nc, in_maps, core_ids=list(range(NCORES)),
        trace=trace, trace_cores=trace_cores)
    full = np.concatenate([res.results[c]["out"] for c in range(NCORES)], axis=0)
    return full, res


def kernel(**inputs) -> np.ndarray:
    out, _ = _run(inputs)
    return out


# revision 53
# speedup vs baseline: 1.0035x; 1.0035x over previous
"""Trainium2 Bass kernel for KL-divergence attention.

Math used (exactly equivalent to the reference model):
  q = x@Wq, k = x@Wk, v = x@Wv
  kl_ij  = sum_h p_i log p_i - p_i . logq_j   (p = softmax(q), logq = log_softmax(k))
  attn   = softmax_j(-kl_ij) = softmax_j(p_i . logq_j)     [neg-entropy cancels]
         = softmax_j(p_i . k_j - lse_j)
  With exp(s - lse_j) = exp(s)/sk_j (sk_j = sum_h exp(k_jh)), the 1/sk_j
  factor is absorbed into the V rows and the softmax-denominator matmul, so
  no log is needed. With e_ij = exp(p_i . k_j), v'_j = 256 v_j / sk_j,
  r'_j = 256 / sk_j:
    out_i = (sum_j e_ij v'_j) / (sum_j e_ij r'_j)

Precision strategy (validated against a numpy simulation of the full cast
chain): the attention output is a near-cancellation (diffuse weights
averaging zero-mean v rows), so per-element quantization noise on e or v
passes straight through to the output — naive fp8 there costs ~2.6e-2 rel
err. Instead the output GEMM is CENTERED: e = 1 + f, so

    num_i = colsum(v') + sum_j f_ij v'_j
    den_i = colsum(r') + sum_j f_ij r'_j

The rank-1 colsum terms are computed exactly from fp16 v' during phase 2
and injected into each PSUM chain with one K=1 matmul; only the small
residual f (|f| ~ 0.3, quantization noise ~5x below e's) is contracted in
fp8 DoubleRow. Full error budget (hardware): ~1.2e-2 vs the 2e-2 gate.

Per-GEMM precisions:
  - q/k projections: fp8e4 DoubleRow (x fp8; W pre-scaled x16 into fp8's
    normal range; q_ps = 16q, undone by the exp's scale)
  - kT: computed DIRECTLY as a second DoubleRow projection (Wk stationary,
    xT moving -> [h, j] layout), replacing per-block PE transposes + casts
  - pairwise scores GEMM: fp8 DoubleRow (pT = 128p, kT = 16k, exp scale
    1/2048; p*128 <= 128 < 240 so no fp8 overflow is possible)
  - v projection: fp16 (v feeds the output linearly — fp8 unacceptable)
  - f residual + output GEMM + denominator: fp8 DoubleRow over the
    centered residual; exp -> fp16 on Scalar, the -1 subtract + fp8 cast
    runs on the otherwise-idle GpSimd engine (SBUF->SBUF)
  - denominator contraction: REVERSED matmuls (lhsT = r' column pair ->
    2-column LDWEIGHTS, ~free) into a [1, 512] PSUM row per i-group,
    transposed back onto partitions with 4 tiny fp32 identity matmuls
  - scores are computed TRANSPOSED (sT[j,i] = kT.T @ pT) so f feeds the
    output GEMM as its stationary operand with no transpose
  - the p transpose runs against diag(128/sq) (fp16), folding the softmax
    normalization and fp8 pre-scale into the transpose matmul
"""

import numpy as np

import concourse.bass as bass
import concourse.tile as tile
from concourse import bacc, mybir
from concourse.bass_utils import run_bass_kernel_spmd
from concourse.masks import make_identity

B, S, D, H = 32, 2048, 512, 512
NCORES = 8
BPC = B // NCORES  # batches per core
P = 128
NB = S // P   # 16 row blocks per batch
ND = D // P   # 4 contraction chunks
NH = H // P   # 4 h chunks
NG = 4        # i groups in phase 3
GW = S // NG  # 512 i columns per group
NPAIR = NB // 2

FP32 = mybir.dt.float32
FP16 = mybir.dt.float16
FP8 = mybir.dt.float8e4
EXP = mybir.ActivationFunctionType.Exp
DR = mybir.MatmulPerfMode.DoubleRow
MUL = mybir.AluOpType.mult
SUB = mybir.AluOpType.subtract

WS = 16.0        # host-side Wq/Wk pre-scale (keeps fp8 W in normal range)
PS = 128.0       # p pre-scale: p*PS <= 128 < 240 (fp8e4 max) always safe
SK_SCALE = 256.0  # v/rsk pre-scale; cancels between numerator/denominator


def _emit(tc):
    # Inputs arrive pre-sharded/pre-laid-out by the host side of kernel():
    # x as [BPC, D, S] in BOTH fp8 (q/k path) and fp16 (v path), each W as
    # [128, ND, H] chunked on the contraction dim (Wq/Wk fp8 pre-scaled by
    # WS, Wv fp16 unscaled).
    nc = tc.nc
    x8 = nc.dram_tensor("x8", [BPC, D, S], FP8, kind="ExternalInput").ap()
    x16 = nc.dram_tensor("x16", [BPC, D, S], FP16, kind="ExternalInput").ap()
    wq = nc.dram_tensor("Wq", [P, ND, H], FP8, kind="ExternalInput").ap()
    wk = nc.dram_tensor("Wk", [P, ND, H], FP8, kind="ExternalInput").ap()
    wv = nc.dram_tensor("Wv", [P, ND, H], FP16, kind="ExternalInput").ap()
    out = nc.dram_tensor("out", [BPC, S, H], FP32, kind="ExternalOutput").ap()

    import contextlib

    with contextlib.ExitStack() as ctx:
        consts = ctx.enter_context(tc.tile_pool(name="consts", bufs=1))
        big = ctx.enter_context(tc.tile_pool(name="big", bufs=1))
        vpool = ctx.enter_context(tc.tile_pool(name="vpool", bufs=2))
        fpool = ctx.enter_context(tc.tile_pool(name="fpool", bufs=2))
        stage = ctx.enter_context(tc.tile_pool(name="stage", bufs=4))
        small = ctx.enter_context(tc.tile_pool(name="small", bufs=4))
        dpool = ctx.enter_context(tc.tile_pool(name="dpool", bufs=2))
        outp = ctx.enter_context(tc.tile_pool(name="outp", bufs=4))
        psp = ctx.enter_context(tc.tile_pool(name="psp", bufs=4, space="PSUM"))

        ident32 = consts.tile([P, P], FP32)
        make_identity(nc, ident32)
        ident16 = consts.tile([P, P], FP16)
        nc.vector.tensor_copy(ident16, ident32)
        ones32 = consts.tile([1, 1], FP32)
        nc.vector.memset(ones32, 1.0)
        ones32c = consts.tile([P, 1], FP32)
        nc.vector.memset(ones32c, 1.0)
        ones16c = consts.tile([P, 1], FP16)
        nc.vector.memset(ones16c, 1.0)
        ones16r = consts.tile([1, GW], FP16)
        nc.vector.memset(ones16r, 1.0)

        # Weights arrive pre-chunked/pre-scaled; straight DMA.
        # wk first on the Sync ring (the opening kT-direct chains need only
        # wk + x8-q0); wq/wv go out on the GpSimd ring in parallel (first
        # needed by the projections, several microseconds in)
        wk_f = consts.tile([P, ND, H], FP8, name="wk_sb")
        nc.sync.dma_start(out=wk_f, in_=wk)
        wq_f = consts.tile([P, ND, H], FP8, name="wq_sb")
        nc.sync.dma_start(out=wq_f, in_=wq)
        wv_f = consts.tile([P, ND, H], FP16, name="wv_sb")

        def alloc_and_dma_x(b):
            # ---- phase 1: xT arrives pre-transposed — straight DMA ----
            xT8 = big.tile([P, ND, S], FP8, tag="xT8", bufs=2, name=f"xT8_{b}")
            xT16 = big.tile([P, ND, S], FP16, tag="xT16", bufs=2,
                            name=f"xT16_{b}")
            xsrc8 = x8[b].rearrange("(c p) s -> p c s", p=P)
            xsrc16 = x16[b].rearrange("(c p) s -> p c s", p=P)
            # x8 lands first (kT-direct + q/k projections need it); x16
            # (v path) follows
            for q4 in range(4):
                cols = slice(q4 * GW, (q4 + 1) * GW)
                nc.sync.dma_start(out=xT8[:, :, cols], in_=xsrc8[:, :, cols])
                if q4 == 0:
                    nc.sync.dma_start(out=xT16[:, :, cols],
                                      in_=xsrc16[:, :, cols])
            for q4 in range(1, 4):
                cols = slice(q4 * GW, (q4 + 1) * GW)
                nc.sync.dma_start(out=xT16[:, :, cols], in_=xsrc16[:, :, cols])
            kT = big.tile([P, NH, S], FP8, tag="kT", bufs=2, name=f"kT_{b}")
            return {"xT8": xT8, "xT16": xT16, "kT": kT}

        def emit_ktd_chain(tiles, jq, hc):
            # kT computed directly: kT[h, j] = sum_d Wk[d, h] xT[d, j]
            jcols = slice(jq * GW, (jq + 1) * GW)
            kt_ps = psp.tile([P, GW], FP32, tag="ps", name="kt_ps")
            for c in range(ND // 2):
                nc.tensor.matmul(
                    kt_ps,
                    wk_f[:, 2 * c:2 * c + 2, hc * P:(hc + 1) * P],
                    tiles["xT8"][:, 2 * c:2 * c + 2, jcols],
                    start=(c == 0), stop=(c == ND // 2 - 1),
                    perf_mode=DR)
            nc.vector.tensor_copy(out=tiles["kT"][:, hc, jcols], in_=kt_ps)

        cur = None
        pending = None
        for b in range(BPC):
            if cur is None:
                cur = alloc_and_dma_x(0)
                nc.sync.dma_start(out=wv_f, in_=wv)
                for jq in range(4):
                    for hc in range(NH):
                        emit_ktd_chain(cur, jq, hc)
            xT8, xT16, kT = cur["xT8"], cur["xT16"], cur["kT"]

            # ---- phase 2: projections, softmax pieces, transposed p ----
            pT = big.tile([P, NH, S], FP8, tag="pT", bufs=2, name=f"pT_{b}")
            # v rows scaled by 256/sk: fp8 copy for the f-GEMM, fp16
            # transients for the exact colsum
            v8_all = vpool.tile([P, NB, H], FP8, tag="v8", name=f"v8_{b}")
            rsk16_all = small.tile([P, NB], FP16, tag="rsk16", bufs=2)
            r8_all = small.tile([P, NB, 16], FP8, tag="r8", bufs=2)
            cs_ps = psp.tile([1, H], FP32, tag="ps2", bufs=2, name="cs_ps")
            eq_tiles = {}
            diag_tiles = {}

            def emit_tr(jb):
                tp = psp.tile([P, H], FP32, tag="ps", name="tpp")
                for hc in range(NH):
                    nc.tensor.matmul(
                        tp[:, hc * P:(hc + 1) * P],
                        eq_tiles[jb][:, hc * P:(hc + 1) * P], diag_tiles[jb],
                        start=True, stop=True)
                nc.vector.tensor_copy(
                    out=pT[:, :, jb * P:(jb + 1) * P],
                    in_=tp.rearrange("p (c f) -> p c f", c=NH))

            for ib in range(NB):
                q_ps = psp.tile([P, H], FP32, tag="ps", name="q_ps")
                k_ps = psp.tile([P, H], FP32, tag="ps", name="k_ps")
                for ps, wt in ((q_ps, wq_f), (k_ps, wk_f)):
                    for c in range(ND // 2):
                        nc.tensor.matmul(
                            ps, xT8[:, 2 * c:2 * c + 2, ib * P:(ib + 1) * P],
                            wt[:, 2 * c:2 * c + 2, :],
                            start=(c == 0), stop=(c == ND // 2 - 1),
                            perf_mode=DR)
                v_ps = psp.tile([P, H], FP32, tag="ps", name="v_ps")
                for dc in range(ND):
                    nc.tensor.matmul(
                        v_ps, xT16[:, dc, ib * P:(ib + 1) * P], wv_f[:, dc, :],
                        start=(dc == 0), stop=(dc == ND - 1))

                eq_sb = stage.tile([P, H], FP16, tag="eq", bufs=5)
                sq = small.tile([P, 1], FP32, tag="sq")
                nc.scalar.activation(eq_sb, q_ps, EXP, scale=1.0 / WS,
                                     accum_out=sq)
                rq = small.tile([P, 1], FP32, tag="rq")
                nc.vector.reciprocal(rq, sq)
                # diag(PS/sq): folds p-normalization and the fp8 pre-scale
                # into the p transpose matmul
                diag = stage.tile([P, P], FP16, tag="diag", bufs=5)
                nc.gpsimd.tensor_scalar(diag, ident16, rq, PS, op0=MUL,
                                        op1=MUL)

                ek_sb = stage.tile([P, H], FP16, tag="ek", bufs=2)
                sk = small.tile([P, 1], FP32, tag="sk")
                nc.scalar.activation(ek_sb, k_ps, EXP, scale=1.0 / WS,
                                     accum_out=sk)
                rsk = small.tile([P, 1], FP32, tag="rsk")
                nc.vector.reciprocal(rsk, sk)
                nc.vector.tensor_scalar_mul(
                    rsk16_all[:, ib:ib + 1], rsk, SK_SCALE)
                nc.vector.tensor_scalar_mul(
                    r8_all[:, ib, 0:1], rsk, SK_SCALE)
                # v' = v * rsk * SK_SCALE: fp16 transient for the exact
                # colsum, fp8 copy (via GpSimd) for the f-GEMM
                v16_sb = stage.tile([P, H], FP16, tag="v16", bufs=3)
                nc.vector.tensor_scalar(
                    v16_sb, v_ps, rsk, SK_SCALE, op0=MUL, op1=MUL)
                nc.gpsimd.tensor_copy(out=v8_all[:, ib, :], in_=v16_sb)
                nc.tensor.matmul(cs_ps, ones16c, v16_sb,
                                 start=(ib == 0), stop=(ib == NB - 1))

                eq_tiles[ib] = eq_sb
                diag_tiles[ib] = diag
                if ib >= 3:
                    emit_tr(ib - 3)

            # exact colsum(r'): free-dim reduce on DVE, then one tiny matmul
            # colsum finalization is deferred into the scores stream (see
            # below) so the phase-2-end Vector backlog never stalls the PE

            # ---- phase 3: scores (transposed, DR), f = e - 1, output ----
            def emit_scores(gp, jbs, fb):
                for jb in jbs:
                    s_ps = psp.tile([P, 2, GW], FP32, tag="ps2", bufs=2,
                                    name="s_ps")
                    for pair in range(2):
                        lhs = kT[:, 2 * pair:2 * pair + 2,
                                 jb * P:(jb + 1) * P]
                        for lg in range(2):
                            ig = 2 * gp + lg
                            nc.tensor.matmul(
                                s_ps[:, lg, :], lhs,
                                pT[:, 2 * pair:2 * pair + 2,
                                   ig * GW:(ig + 1) * GW],
                                start=(pair == 0), stop=(pair == 1),
                                perf_mode=DR)
                    e16 = stage.tile([P, 2 * GW], FP16, tag="e16", bufs=4)
                    nc.scalar.activation(
                        e16, s_ps.rearrange("p a f -> p (a f)"), EXP,
                        scale=1.0 / (PS * WS))
                    nc.vector.tensor_scalar(
                        fb[:, jb, :], e16, 1.0, None, op0=SUB)

            def emit_den(gp, fb):
                # denominators: colsum(r') broadcast + reversed DR matmuls
                # over the f residual, then the [1, 512] PSUM row is
                # transposed back onto partitions with 4 tiny fp32 matmuls.
                # both den chains first, then the rl transposes — the
                # PSUM->SBUF copy of den(lg0) hides under den(lg1)'s chain
                den_sbs = {}
                for lg in range(2):
                    den_ps = psp.tile([1, GW], FP32, tag="ps", name="den_ps")
                    nc.tensor.matmul(den_ps, csr_sb, ones16r,
                                     start=True, stop=False)
                    for t in range(NPAIR):
                        nc.tensor.matmul(
                            den_ps, r8_all[:, 2 * t:2 * t + 2, 0:1],
                            fb[:, 2 * t:2 * t + 2, lg * GW:(lg + 1) * GW],
                            start=False, stop=(t == NPAIR - 1),
                            perf_mode=DR)
                    den_sb = dpool.tile([1, GW], FP32, tag="den")
                    nc.vector.tensor_copy(den_sb, den_ps)
                    den_sbs[lg] = den_sb
                rl_sbs = {}
                for lg in range(2):
                    rl_ps = psp.tile([P, NG], FP32, tag="ps", name="rl_ps")
                    for il in range(NG):
                        nc.tensor.matmul(
                            rl_ps[:, il:il + 1],
                            den_sbs[lg][0:1, il * P:(il + 1) * P], ones32,
                            start=True, stop=True)
                    rl_sb = small.tile([P, NG], FP32, tag="rl", bufs=4)
                    nc.vector.reciprocal(rl_sb, rl_ps)
                    rl_sbs[lg] = rl_sb
                return rl_sbs

            def emit_out_chain(st, lg, il):
                ig = 2 * st["gp"] + lg
                ib = ig * NG + il
                o_ps = psp.tile([P, H], FP32, tag="ps", name="o_ps")
                nc.tensor.matmul(o_ps, ones16r[0:1, 0:P], st["cs"],
                                 start=True, stop=False)
                for t in range(NPAIR):
                    nc.tensor.matmul(
                        o_ps,
                        st["fb"][:, 2 * t:2 * t + 2,
                                 lg * GW + il * P:lg * GW + (il + 1) * P],
                        st["v8"][:, 2 * t:2 * t + 2, :],
                        start=False, stop=(t == NPAIR - 1),
                        perf_mode=DR)
                o_sb = outp.tile([P, H], FP32, tag="o")
                nc.vector.tensor_scalar_mul(
                    o_sb, o_ps, st["rl"][lg][:, il:il + 1])
                # output stores go out on the GpSimd queue so the next
                # batch's input loads (Sync queue) never queue behind them
                nc.gpsimd.dma_start(
                    out=out[st["b"], ib * P:(ib + 1) * P, :], in_=o_sb)

            # The scores loops are Scalar-bound (the PE needs ~0.9us per
            # row block but the exp takes ~1.2us), so PE-pure work is
            # interleaved into them to fill the bubbles:
            #   group 0: the NEXT batch's kT-direct chains (its x DMAs are
            #            prefetched here, overlapping this batch's compute)
            #            plus the PREVIOUS batch's deferred group-1 output
            #   group 1: group-0's output chains
            # The first 13 gp0 blocks are hoisted in front of the tail p
            # transposes so the latter never stall the PE on the exp
            # backlog; den(0) trails two gp1 blocks to hide the wait on
            # gp0's last f-subtract.
            nxt = alloc_and_dma_x(b + 1) if b + 1 < BPC else None
            ktd_todo = [(jq, hc) for jq in range(4) for hc in range(NH)] \
                if nxt is not None else []
            fb0 = fpool.tile([P, NB, 2 * GW], FP8, tag="f", name=f"f0_{b}")
            for jb in range(NB - 3):
                emit_scores(0, [jb], fb0)
                if ktd_todo:
                    emit_ktd_chain(nxt, *ktd_todo.pop(0))
                if jb == 0:
                    # deferred colsum(r') finalization: free-dim reduce on
                    # DVE, then one tiny matmul for the cross-partition sum
                    rsum = small.tile([P, 1], FP32, tag="rsum", bufs=2)
                    nc.vector.tensor_reduce(rsum, rsk16_all,
                                            mybir.AxisListType.X,
                                            mybir.AluOpType.add)
                    csr_ps = psp.tile([1, 1], FP32, tag="ps", name="csr_ps")
                    nc.tensor.matmul(csr_ps, rsum, ones32c,
                                     start=True, stop=True)
                    csr_sb = small.tile([1, 1], FP16, tag="csr", bufs=2)
                    nc.vector.tensor_copy(csr_sb, csr_ps)
                elif jb == 1:
                    cs_sb = dpool.tile([1, H], FP16, tag="cs")
                    nc.vector.tensor_copy(cs_sb, cs_ps)
            emit_tr(NB - 3)
            emit_tr(NB - 2)
            emit_tr(NB - 1)
            for jb in range(NB - 3, NB):
                emit_scores(0, [jb], fb0)
                if ktd_todo:
                    emit_ktd_chain(nxt, *ktd_todo.pop(0))
            fb1 = fpool.tile([P, NB, 2 * GW], FP8, tag="f", name=f"f1_{b}")
            emit_scores(1, [0], fb1)
            while ktd_todo:
                emit_ktd_chain(nxt, *ktd_todo.pop(0))
            emit_scores(1, [1], fb1)
            rl0 = emit_den(0, fb0)
            st0 = {"b": b, "gp": 0, "fb": fb0, "v8": v8_all, "rl": rl0,
                   "cs": cs_sb}
            for jb in range(2, NB):
                emit_scores(1, [jb], fb1)
                if jb % 2 == 1:
                    c = jb // 2
                    emit_out_chain(st0, c // NG, c % NG)
            emit_out_chain(st0, 0, 0)
            rl1 = emit_den(1, fb1)
            st1 = {"b": b, "gp": 1, "fb": fb1, "v8": v8_all, "rl": rl1,
                   "cs": cs_sb}
            for lg in range(2):
                for il in range(NG):
                    emit_out_chain(st1, lg, il)
            cur = nxt


_NC_CACHE = {}


def _get_nc():
    if "nc" not in _NC_CACHE:
        nc = bacc.Bacc("TRN2", target_bir_lowering=False, debug=False)
        with tile.TileContext(nc) as tc:
            _emit(tc)
        nc.compile()
        _NC_CACHE["nc"] = nc
    return _NC_CACHE["nc"]


_NP8 = mybir.dt.np(FP8)


def _prep_w(w, scale, npdt):
    # [D, H] fp32 -> [128, ND, H] chunked on the contraction dim, *scale
    w = np.asarray(w, dtype=np.float32).reshape(ND, P, H).transpose(1, 0, 2)
    return np.ascontiguousarray(w * scale).astype(npdt)


def _run(inputs, trace=False, trace_cores=None):
    nc = _get_nc()
    x = np.asarray(inputs["x"], dtype=np.float32)
    wq = _prep_w(inputs["Wq"], WS, _NP8)
    wk = _prep_w(inputs["Wk"], WS, _NP8)
    wv = _prep_w(inputs["Wv"], 1.0, np.float16)
    in_maps = []
    for c in range(NCORES):
        xs = x[c * BPC:(c + 1) * BPC]
        xt = np.ascontiguousarray(xs.transpose(0, 2, 1))
        in_maps.append({
            "x8": xt.astype(_NP8), "x16": xt.astype(np.float16),
            "Wq": wq, "Wk": wk, "Wv": wv,
        })
    res = run_bass_kernel_spmd(
        nc, in_maps, core_ids=list(range(NCORES)),
        trace=trace, trace_cores=trace_cores)
    full = np.concatenate([res.results[c]["out"] for c in range(NCORES)], axis=0)
    return full, res


def kernel(**inputs) -> np.ndarray:
    out, _ = _run(inputs)
    return out


# revision 55
# speedup vs baseline: 1.0086x; 1.0051x over previous
"""Trainium2 Bass kernel for KL-divergence attention.

Math used (exactly equivalent to the reference model):
  q = x@Wq, k = x@Wk, v = x@Wv
  kl_ij  = sum_h p_i log p_i - p_i . logq_j   (p = softmax(q), logq = log_softmax(k))
  attn   = softmax_j(-kl_ij) = softmax_j(p_i . logq_j)     [neg-entropy cancels]
         = softmax_j(p_i . k_j - lse_j)
  With exp(s - lse_j) = exp(s)/sk_j (sk_j = sum_h exp(k_jh)), the 1/sk_j
  factor is absorbed into the V rows and the softmax-denominator matmul, so
  no log is needed. With e_ij = exp(p_i . k_j), v'_j = 256 v_j / sk_j,
  r'_j = 256 / sk_j:
    out_i = (sum_j e_ij v'_j) / (sum_j e_ij r'_j)

Precision strategy (validated against a numpy simulation of the full cast
chain): the attention output is a near-cancellation (diffuse weights
averaging zero-mean v rows), so per-element quantization noise on e or v
passes straight through to the output — naive fp8 there costs ~2.6e-2 rel
err. Instead the output GEMM is CENTERED: e = 1 + f, so

    num_i = colsum(v') + sum_j f_ij v'_j
    den_i = colsum(r') + sum_j f_ij r'_j

The rank-1 colsum terms are computed exactly from fp16 v' during phase 2
and injected into each PSUM chain with one K=1 matmul; only the small
residual f (|f| ~ 0.3, quantization noise ~5x below e's) is contracted in
fp8 DoubleRow. Full error budget (hardware): ~1.2e-2 vs the 2e-2 gate.

Per-GEMM precisions:
  - q/k projections: fp8e4 DoubleRow (x fp8; W pre-scaled x16 into fp8's
    normal range; q_ps = 16q, undone by the exp's scale)
  - kT: computed DIRECTLY as a second DoubleRow projection (Wk stationary,
    xT moving -> [h, j] layout), replacing per-block PE transposes + casts
  - pairwise scores GEMM: fp8 DoubleRow (pT = 128p, kT = 16k, exp scale
    1/2048; p*128 <= 128 < 240 so no fp8 overflow is possible)
  - v projection: fp16 (v feeds the output linearly — fp8 unacceptable)
  - f residual + output GEMM + denominator: fp8 DoubleRow over the
    centered residual; exp -> fp16 on Scalar, the -1 subtract + fp8 cast
    on Vector
  - denominator contraction: REVERSED matmuls (lhsT = r' column pair ->
    2-column LDWEIGHTS, ~free) into a [1, 512] PSUM row per i-group,
    transposed back onto partitions with 4 tiny fp32 identity matmuls
  - scores are computed TRANSPOSED (sT[j,i] = kT.T @ pT) so f feeds the
    output GEMM as its stationary operand with no transpose
  - the p transpose runs against diag(128/sq) (fp16), folding the softmax
    normalization and fp8 pre-scale into the transpose matmul

Scheduling (the kernel is PE-issue-bound at ~215ns per matmul; every
other engine is load-balanced under that): the scores phases alone are
Scalar-bound (~0.9us of PE work per row block vs ~1.2us of exp), so
PE-pure work is interleaved into them — the NEXT batch's kT-direct
chains (whose x DMAs are prefetched a phase early) into group-0's loop,
and group-0's output chains into group-1's loop. The diag build and the
v8 cast run on GpSimd; the colsum finalization is deferred past the
first score block so the phase-2-end Vector backlog never stalls the PE;
output stores ride the GpSimd DMA queue so input prefetches on the Sync
queue are never stuck behind them.
"""

import numpy as np

import concourse.bass as bass
import concourse.tile as tile
from concourse import bacc, mybir
from concourse.bass_utils import run_bass_kernel_spmd
from concourse.masks import make_identity

B, S, D, H = 32, 2048, 512, 512
NCORES = 8
BPC = B // NCORES  # batches per core
P = 128
NB = S // P   # 16 row blocks per batch
ND = D // P   # 4 contraction chunks
NH = H // P   # 4 h chunks
NG = 4        # i groups in phase 3
GW = S // NG  # 512 i columns per group
NPAIR = NB // 2

FP32 = mybir.dt.float32
FP16 = mybir.dt.float16
FP8 = mybir.dt.float8e4
EXP = mybir.ActivationFunctionType.Exp
DR = mybir.MatmulPerfMode.DoubleRow
MUL = mybir.AluOpType.mult
SUB = mybir.AluOpType.subtract

WS = 16.0        # host-side Wq/Wk pre-scale (keeps fp8 W in normal range)
PS = 128.0       # p pre-scale: p*PS <= 128 < 240 (fp8e4 max) always safe
SK_SCALE = 256.0  # v/rsk pre-scale; cancels between numerator/denominator


def _emit(tc):
    # Inputs arrive pre-sharded/pre-laid-out by the host side of kernel():
    # x as [BPC, D, S] in BOTH fp8 (q/k path) and fp16 (v path), each W as
    # [128, ND, H] chunked on the contraction dim (Wq/Wk fp8 pre-scaled by
    # WS, Wv fp16 unscaled).
    nc = tc.nc
    x8 = nc.dram_tensor("x8", [BPC, D, S], FP8, kind="ExternalInput").ap()
    x16 = nc.dram_tensor("x16", [BPC, D, S], FP16, kind="ExternalInput").ap()
    wq = nc.dram_tensor("Wq", [P, ND, H], FP8, kind="ExternalInput").ap()
    wk = nc.dram_tensor("Wk", [P, ND, H], FP8, kind="ExternalInput").ap()
    wv = nc.dram_tensor("Wv", [P, ND, H], FP16, kind="ExternalInput").ap()
    out = nc.dram_tensor("out", [BPC, S, H], FP32, kind="ExternalOutput").ap()

    import contextlib

    with contextlib.ExitStack() as ctx:
        consts = ctx.enter_context(tc.tile_pool(name="consts", bufs=1))
        big = ctx.enter_context(tc.tile_pool(name="big", bufs=1))
        vpool = ctx.enter_context(tc.tile_pool(name="vpool", bufs=2))
        fpool = ctx.enter_context(tc.tile_pool(name="fpool", bufs=2))
        stage = ctx.enter_context(tc.tile_pool(name="stage", bufs=4))
        small = ctx.enter_context(tc.tile_pool(name="small", bufs=4))
        dpool = ctx.enter_context(tc.tile_pool(name="dpool", bufs=2))
        outp = ctx.enter_context(tc.tile_pool(name="outp", bufs=4))
        psp = ctx.enter_context(tc.tile_pool(name="psp", bufs=4, space="PSUM"))

        ident32 = consts.tile([P, P], FP32)
        make_identity(nc, ident32)
        ident16 = consts.tile([P, P], FP16)
        nc.vector.tensor_copy(ident16, ident32)
        ones32 = consts.tile([1, 1], FP32)
        nc.vector.memset(ones32, 1.0)
        ones32c = consts.tile([P, 1], FP32)
        nc.vector.memset(ones32c, 1.0)
        ones16c = consts.tile([P, 1], FP16)
        nc.vector.memset(ones16c, 1.0)
        ones16r = consts.tile([1, GW], FP16)
        nc.vector.memset(ones16r, 1.0)

        # Weights arrive pre-chunked/pre-scaled; straight DMA.
        # wk first on the Sync ring (the opening kT-direct chains need only
        # wk + x8-q0); wq/wv go out on the GpSimd ring in parallel (first
        # needed by the projections, several microseconds in)
        wk_f = consts.tile([P, ND, H], FP8, name="wk_sb")
        nc.sync.dma_start(out=wk_f, in_=wk)
        wq_f = consts.tile([P, ND, H], FP8, name="wq_sb")
        nc.sync.dma_start(out=wq_f, in_=wq)
        wv_f = consts.tile([P, ND, H], FP16, name="wv_sb")

        def alloc_and_dma_x(b):
            # ---- phase 1: xT arrives pre-transposed — straight DMA ----
            xT8 = big.tile([P, ND, S], FP8, tag="xT8", bufs=2, name=f"xT8_{b}")
            xT16 = big.tile([P, ND, S], FP16, tag="xT16", bufs=2,
                            name=f"xT16_{b}")
            xsrc8 = x8[b].rearrange("(c p) s -> p c s", p=P)
            xsrc16 = x16[b].rearrange("(c p) s -> p c s", p=P)
            # x8 lands first (kT-direct + q/k projections need it); x16
            # (v path) follows
            for q4 in range(4):
                cols = slice(q4 * GW, (q4 + 1) * GW)
                nc.sync.dma_start(out=xT8[:, :, cols], in_=xsrc8[:, :, cols])
                if q4 == 0:
                    nc.sync.dma_start(out=xT16[:, :, cols],
                                      in_=xsrc16[:, :, cols])
            for q4 in range(1, 4):
                cols = slice(q4 * GW, (q4 + 1) * GW)
                nc.sync.dma_start(out=xT16[:, :, cols], in_=xsrc16[:, :, cols])
            kT = big.tile([P, NH, S], FP8, tag="kT", bufs=2, name=f"kT_{b}")
            return {"xT8": xT8, "xT16": xT16, "kT": kT}

        def emit_ktd_chain(tiles, jq, hc):
            # kT computed directly: kT[h, j] = sum_d Wk[d, h] xT[d, j]
            jcols = slice(jq * GW, (jq + 1) * GW)
            kt_ps = psp.tile([P, GW], FP32, tag="ps", name="kt_ps")
            for c in range(ND // 2):
                nc.tensor.matmul(
                    kt_ps,
                    wk_f[:, 2 * c:2 * c + 2, hc * P:(hc + 1) * P],
                    tiles["xT8"][:, 2 * c:2 * c + 2, jcols],
                    start=(c == 0), stop=(c == ND // 2 - 1),
                    perf_mode=DR)
            nc.vector.tensor_copy(out=tiles["kT"][:, hc, jcols], in_=kt_ps)

        cur = None
        for b in range(BPC):
            if cur is None:
                cur = alloc_and_dma_x(0)
                nc.sync.dma_start(out=wv_f, in_=wv)
                for jq in range(4):
                    for hc in range(NH):
                        emit_ktd_chain(cur, jq, hc)
            xT8, xT16, kT = cur["xT8"], cur["xT16"], cur["kT"]

            # ---- phase 2: projections, softmax pieces, transposed p ----
            pT = big.tile([P, NH, S], FP8, tag="pT", bufs=2, name=f"pT_{b}")
            # v rows scaled by 256/sk: fp8 copy for the f-GEMM, fp16
            # transients for the exact colsum
            v8_all = vpool.tile([P, NB, H], FP8, tag="v8", name=f"v8_{b}")
            rsk16_all = small.tile([P, NB], FP16, tag="rsk16", bufs=2)
            r8_all = small.tile([P, NB, 16], FP8, tag="r8", bufs=2)
            cs_ps = psp.tile([1, H], FP32, tag="ps2", bufs=2, name="cs_ps")
            eq_tiles = {}
            diag_tiles = {}

            def emit_tr(jb):
                tp = psp.tile([P, H], FP32, tag="ps", name="tpp")
                for hc in range(NH):
                    nc.tensor.matmul(
                        tp[:, hc * P:(hc + 1) * P],
                        eq_tiles[jb][:, hc * P:(hc + 1) * P], diag_tiles[jb],
                        start=True, stop=True)
                nc.vector.tensor_copy(
                    out=pT[:, :, jb * P:(jb + 1) * P],
                    in_=tp.rearrange("p (c f) -> p c f", c=NH))

            for ib in range(NB):
                q_ps = psp.tile([P, H], FP32, tag="ps", name="q_ps")
                k_ps = psp.tile([P, H], FP32, tag="ps", name="k_ps")
                for ps, wt in ((q_ps, wq_f), (k_ps, wk_f)):
                    for c in range(ND // 2):
                        nc.tensor.matmul(
                            ps, xT8[:, 2 * c:2 * c + 2, ib * P:(ib + 1) * P],
                            wt[:, 2 * c:2 * c + 2, :],
                            start=(c == 0), stop=(c == ND // 2 - 1),
                            perf_mode=DR)
                v_ps = psp.tile([P, H], FP32, tag="ps", name="v_ps")
                for dc in range(ND):
                    nc.tensor.matmul(
                        v_ps, xT16[:, dc, ib * P:(ib + 1) * P], wv_f[:, dc, :],
                        start=(dc == 0), stop=(dc == ND - 1))

                eq_sb = stage.tile([P, H], FP16, tag="eq", bufs=5)
                sq = small.tile([P, 1], FP32, tag="sq")
                nc.scalar.activation(eq_sb, q_ps, EXP, scale=1.0 / WS,
                                     accum_out=sq)
                rq = small.tile([P, 1], FP32, tag="rq")
                nc.vector.reciprocal(rq, sq)
                # diag(PS/sq): folds p-normalization and the fp8 pre-scale
                # into the p transpose matmul
                diag = stage.tile([P, P], FP16, tag="diag", bufs=5)
                nc.gpsimd.tensor_scalar(diag, ident16, rq, PS, op0=MUL,
                                        op1=MUL)

                ek_sb = stage.tile([P, H], FP16, tag="ek", bufs=2)
                sk = small.tile([P, 1], FP32, tag="sk")
                nc.scalar.activation(ek_sb, k_ps, EXP, scale=1.0 / WS,
                                     accum_out=sk)
                rsk = small.tile([P, 1], FP32, tag="rsk")
                nc.vector.reciprocal(rsk, sk)
                nc.vector.tensor_scalar_mul(
                    rsk16_all[:, ib:ib + 1], rsk, SK_SCALE)
                nc.vector.tensor_scalar_mul(
                    r8_all[:, ib, 0:1], rsk, SK_SCALE)
                # v' = v * rsk * SK_SCALE: fp16 transient for the exact
                # colsum, fp8 copy (via GpSimd) for the f-GEMM
                v16_sb = stage.tile([P, H], FP16, tag="v16", bufs=3)
                nc.vector.tensor_scalar(
                    v16_sb, v_ps, rsk, SK_SCALE, op0=MUL, op1=MUL)
                nc.gpsimd.tensor_copy(out=v8_all[:, ib, :], in_=v16_sb)
                nc.tensor.matmul(cs_ps, ones16c, v16_sb,
                                 start=(ib == 0), stop=(ib == NB - 1))

                eq_tiles[ib] = eq_sb
                diag_tiles[ib] = diag
                if ib >= 3:
                    emit_tr(ib - 3)

            # exact colsum(r'): free-dim reduce on DVE, then one tiny matmul
            # colsum finalization is deferred into the scores stream (see
            # below) so the phase-2-end Vector backlog never stalls the PE

            # ---- phase 3: scores (transposed, DR), f = e - 1, output ----
            def emit_scores(gp, jbs, fb):
                for jb in jbs:
                    s_ps = psp.tile([P, 2, GW], FP32, tag="ps2", bufs=2,
                                    name="s_ps")
                    for pair in range(2):
                        lhs = kT[:, 2 * pair:2 * pair + 2,
                                 jb * P:(jb + 1) * P]
                        for lg in range(2):
                            ig = 2 * gp + lg
                            nc.tensor.matmul(
                                s_ps[:, lg, :], lhs,
                                pT[:, 2 * pair:2 * pair + 2,
                                   ig * GW:(ig + 1) * GW],
                                start=(pair == 0), stop=(pair == 1),
                                perf_mode=DR)
                    e16 = stage.tile([P, 2 * GW], FP16, tag="e16", bufs=4)
                    nc.scalar.activation(
                        e16, s_ps.rearrange("p a f -> p (a f)"), EXP,
                        scale=1.0 / (PS * WS))
                    nc.vector.tensor_scalar(
                        fb[:, jb, :], e16, 1.0, None, op0=SUB)

            def emit_den(gp, fb):
                # denominators: colsum(r') broadcast + reversed DR matmuls
                # over the f residual, then the [1, 512] PSUM row is
                # transposed back onto partitions with 4 tiny fp32 matmuls.
                # both den chains first, then the rl transposes — the
                # PSUM->SBUF copy of den(lg0) hides under den(lg1)'s chain
                den_sbs = {}
                for lg in range(2):
                    den_ps = psp.tile([1, GW], FP32, tag="ps", name="den_ps")
                    nc.tensor.matmul(den_ps, csr_sb, ones16r,
                                     start=True, stop=False)
                    for t in range(NPAIR):
                        nc.tensor.matmul(
                            den_ps, r8_all[:, 2 * t:2 * t + 2, 0:1],
                            fb[:, 2 * t:2 * t + 2, lg * GW:(lg + 1) * GW],
                            start=False, stop=(t == NPAIR - 1),
                            perf_mode=DR)
                    den_sb = dpool.tile([1, GW], FP32, tag="den")
                    nc.vector.tensor_copy(den_sb, den_ps)
                    den_sbs[lg] = den_sb
                rl_sbs = {}
                for lg in range(2):
                    rl_ps = psp.tile([P, NG], FP32, tag="ps", name="rl_ps")
                    for il in range(NG):
                        nc.tensor.matmul(
                            rl_ps[:, il:il + 1],
                            den_sbs[lg][0:1, il * P:(il + 1) * P], ones32,
                            start=True, stop=True)
                    rl_sb = small.tile([P, NG], FP32, tag="rl", bufs=4)
                    nc.vector.reciprocal(rl_sb, rl_ps)
                    rl_sbs[lg] = rl_sb
                return rl_sbs

            def emit_out_chain(st, lg, il):
                ig = 2 * st["gp"] + lg
                ib = ig * NG + il
                o_ps = psp.tile([P, H], FP32, tag="ps", name="o_ps")
                nc.tensor.matmul(o_ps, ones16r[0:1, 0:P], st["cs"],
                                 start=True, stop=False)
                for t in range(NPAIR):
                    nc.tensor.matmul(
                        o_ps,
                        st["fb"][:, 2 * t:2 * t + 2,
                                 lg * GW + il * P:lg * GW + (il + 1) * P],
                        st["v8"][:, 2 * t:2 * t + 2, :],
                        start=False, stop=(t == NPAIR - 1),
                        perf_mode=DR)
                o_sb = outp.tile([P, H], FP32, tag="o")
                nc.vector.tensor_scalar_mul(
                    o_sb, o_ps, st["rl"][lg][:, il:il + 1])
                # output stores go out on the GpSimd queue so the next
                # batch's input loads (Sync queue) never queue behind them
                nc.gpsimd.dma_start(
                    out=out[st["b"], ib * P:(ib + 1) * P, :], in_=o_sb)

            # The scores loops are Scalar-bound (the PE needs ~0.9us per
            # row block but the exp takes ~1.2us), so PE-pure work is
            # interleaved into them to fill the bubbles:
            #   group 0: the NEXT batch's kT-direct chains (its x DMAs are
            #            prefetched here, overlapping this batch's compute)
            #            plus the PREVIOUS batch's deferred group-1 output
            #   group 1: group-0's output chains
            # The first 13 gp0 blocks are hoisted in front of the tail p
            # transposes so the latter never stall the PE on the exp
            # backlog; den(0) trails two gp1 blocks to hide the wait on
            # gp0's last f-subtract.
            nxt = alloc_and_dma_x(b + 1) if b + 1 < BPC else None
            ktd_todo = [(jq, hc) for jq in range(4) for hc in range(NH)] \
                if nxt is not None else []
            fb0 = fpool.tile([P, NB, 2 * GW], FP8, tag="f", name=f"f0_{b}")
            for jb in range(NB - 3):
                emit_scores(0, [jb], fb0)
                if ktd_todo:
                    emit_ktd_chain(nxt, *ktd_todo.pop(0))
                if jb == 0:
                    # deferred colsum(r') finalization: free-dim reduce on
                    # DVE, then one tiny matmul for the cross-partition sum
                    rsum = small.tile([P, 1], FP32, tag="rsum", bufs=2)
                    nc.vector.tensor_reduce(rsum, rsk16_all,
                                            mybir.AxisListType.X,
                                            mybir.AluOpType.add)
                    csr_ps = psp.tile([1, 1], FP32, tag="ps", name="csr_ps")
                    nc.tensor.matmul(csr_ps, rsum, ones32c,
                                     start=True, stop=True)
                    csr_sb = small.tile([1, 1], FP16, tag="csr", bufs=2)
                    nc.vector.tensor_copy(csr_sb, csr_ps)
                elif jb == 1:
                    cs_sb = dpool.tile([1, H], FP16, tag="cs")
                    nc.vector.tensor_copy(cs_sb, cs_ps)
            emit_tr(NB - 3)
            emit_tr(NB - 2)
            emit_tr(NB - 1)
            for jb in range(NB - 3, NB):
                emit_scores(0, [jb], fb0)
                if ktd_todo:
                    emit_ktd_chain(nxt, *ktd_todo.pop(0))
            fb1 = fpool.tile([P, NB, 2 * GW], FP8, tag="f", name=f"f1_{b}")
            emit_scores(1, [0], fb1)
            while ktd_todo:
                emit_ktd_chain(nxt, *ktd_todo.pop(0))
            emit_scores(1, [1], fb1)
            rl0 = emit_den(0, fb0)
            st0 = {"b": b, "gp": 0, "fb": fb0, "v8": v8_all, "rl": rl0,
                   "cs": cs_sb}
            for jb in range(2, NB):
                emit_scores(1, [jb], fb1)
                if jb % 2 == 1:
                    c = jb // 2
                    emit_out_chain(st0, c // NG, c % NG)
            emit_out_chain(st0, 0, 0)
            rl1 = emit_den(1, fb1)
            st1 = {"b": b, "gp": 1, "fb": fb1, "v8": v8_all, "rl": rl1,
                   "cs": cs_sb}
            for lg in range(2):
                for il in range(NG):
                    emit_out_chain(st1, lg, il)
            cur = nxt


_NC_CACHE = {}


def _get_nc():
    if "nc" not in _NC_CACHE:
        nc = bacc.Bacc("TRN2", target_bir_lowering=False, debug=False)
        with tile.TileContext(nc) as tc:
            _emit(tc)
        nc.compile()
        _NC_CACHE["nc"] = nc
    return _NC_CACHE["nc"]


_NP8 = mybir.dt.np(FP8)


def _prep_w(w, scale, npdt):
    # [D, H] fp32 -> [128, ND, H] chunked on the contraction dim, *scale
    w = np.asarray(w, dtype=np.float32).reshape(ND, P, H).transpose(1, 0, 2)
    return np.ascontiguousarray(w * scale).astype(npdt)


def _run(inputs, trace=False, trace_cores=None):
    nc = _get_nc()
    x = np.asarray(inputs["x"], dtype=np.float32)
    wq = _prep_w(inputs["Wq"], WS, _NP8)
    wk = _prep_w(inputs["Wk"], WS, _NP8)
    wv = _prep_w(inputs["Wv"], 1.0, np.float16)
    in_maps = []
    for c in range(NCORES):
        xs = x[c * BPC:(c + 1) * BPC]
        xt = np.ascontiguousarray(xs.transpose(0, 2, 1))
        in_maps.append({
            "x8": xt.astype(_NP8), "x16": xt.astype(np.float16),
            "Wq": wq, "Wk": wk, "Wv": wv,
        })
    res = run_bass_kernel_spmd(
        nc, in_maps, core_ids=list(range(NCORES)),
        trace=trace, trace_cores=trace_cores)
    full = np.concatenate([res.results[c]["out"] for c in range(NCORES)], axis=0)
    return full, res


def kernel(**inputs) -> np.ndarray:
    out, _ = _run(inputs)
    return out


# revision 58
# speedup vs baseline: 1.0112x; 1.0026x over previous
"""Trainium2 Bass kernel for KL-divergence attention.

Math used (exactly equivalent to the reference model):
  q = x@Wq, k = x@Wk, v = x@Wv
  kl_ij  = sum_h p_i log p_i - p_i . logq_j   (p = softmax(q), logq = log_softmax(k))
  attn   = softmax_j(-kl_ij) = softmax_j(p_i . logq_j)     [neg-entropy cancels]
         = softmax_j(p_i . k_j - lse_j)
  With exp(s - lse_j) = exp(s)/sk_j (sk_j = sum_h exp(k_jh)), the 1/sk_j
  factor is absorbed into the V rows and the softmax-denominator matmul, so
  no log is needed. With e_ij = exp(p_i . k_j), v'_j = 256 v_j / sk_j,
  r'_j = 256 / sk_j:
    out_i = (sum_j e_ij v'_j) / (sum_j e_ij r'_j)

Precision strategy (validated against a numpy simulation of the full cast
chain): the attention output is a near-cancellation (diffuse weights
averaging zero-mean v rows), so per-element quantization noise on e or v
passes straight through to the output — naive fp8 there costs ~2.6e-2 rel
err. Instead the output GEMM is CENTERED: e = 1 + f, so

    num_i = colsum(v') + sum_j f_ij v'_j
    den_i = colsum(r') + sum_j f_ij r'_j

The rank-1 colsum terms are computed exactly from fp16 v' during phase 2
and injected into each PSUM chain with one K=1 matmul; only the small
residual f (|f| ~ 0.3, quantization noise ~5x below e's) is contracted in
fp8 DoubleRow. Full error budget (hardware): ~1.2e-2 vs the 2e-2 gate.

Per-GEMM precisions:
  - q/k projections: fp8e4 DoubleRow (x fp8; W pre-scaled x16 into fp8's
    normal range; q_ps = 16q, undone by the exp's scale)
  - kT: computed DIRECTLY as a second DoubleRow projection (Wk stationary,
    xT moving -> [h, j] layout), replacing per-block PE transposes + casts
  - pairwise scores GEMM: fp8 DoubleRow (pT = 128p, kT = 16k, exp scale
    1/2048; p*128 <= 128 < 240 so no fp8 overflow is possible)
  - v projection: fp16 (v feeds the output linearly — fp8 unacceptable)
  - f residual + output GEMM + denominator: fp8 DoubleRow over the
    centered residual; exp -> fp16 on Scalar, the -1 subtract + fp8 cast
    on Vector
  - denominator contraction: REVERSED matmuls (lhsT = r' column pair ->
    2-column LDWEIGHTS, ~free) into a [1, 512] PSUM row per i-group,
    transposed back onto partitions with 4 tiny fp32 identity matmuls
  - scores are computed TRANSPOSED (sT[j,i] = kT.T @ pT) so f feeds the
    output GEMM as its stationary operand with no transpose
  - the p transpose runs against diag(128/sq) (fp16), folding the softmax
    normalization and fp8 pre-scale into the transpose matmul

Scheduling (the kernel is PE-issue-bound at ~215ns per matmul; every
other engine is load-balanced under that): the scores phases alone are
Scalar-bound (~0.9us of PE work per row block vs ~1.2us of exp), so
PE-pure work is interleaved into them — the NEXT batch's kT-direct
chains (whose x DMAs are prefetched a phase early) into group-0's loop,
and group-0's output chains into group-1's loop. The diag build and the
v8 cast run on GpSimd; the colsum finalization is deferred past the
first score block so the phase-2-end Vector backlog never stalls the PE;
output stores ride the GpSimd DMA queue so input prefetches on the Sync
queue are never stuck behind them.
"""

import numpy as np

import concourse.bass as bass
import concourse.tile as tile
from concourse import bacc, mybir
from concourse.bass_utils import run_bass_kernel_spmd
from concourse.masks import make_identity

B, S, D, H = 32, 2048, 512, 512
NCORES = 8
BPC = B // NCORES  # batches per core
P = 128
NB = S // P   # 16 row blocks per batch
ND = D // P   # 4 contraction chunks
NH = H // P   # 4 h chunks
NG = 4        # i groups in phase 3
GW = S // NG  # 512 i columns per group
NPAIR = NB // 2

FP32 = mybir.dt.float32
FP16 = mybir.dt.float16
FP8 = mybir.dt.float8e4
EXP = mybir.ActivationFunctionType.Exp
DR = mybir.MatmulPerfMode.DoubleRow
MUL = mybir.AluOpType.mult
SUB = mybir.AluOpType.subtract

WS = 16.0        # host-side Wq/Wk pre-scale (keeps fp8 W in normal range)
PS = 128.0       # p pre-scale: p*PS <= 128 < 240 (fp8e4 max) always safe
SK_SCALE = 256.0  # v/rsk pre-scale; cancels between numerator/denominator


def _emit(tc):
    # Inputs arrive pre-sharded/pre-laid-out by the host side of kernel():
    # x as [BPC, D, S] in BOTH fp8 (q/k path) and fp16 (v path), each W as
    # [128, ND, H] chunked on the contraction dim (Wq/Wk fp8 pre-scaled by
    # WS, Wv fp16 unscaled).
    nc = tc.nc
    x8 = nc.dram_tensor("x8", [BPC, D, S], FP8, kind="ExternalInput").ap()
    x16 = nc.dram_tensor("x16", [BPC, D, S], FP16, kind="ExternalInput").ap()
    wq = nc.dram_tensor("Wq", [P, ND, H], FP8, kind="ExternalInput").ap()
    wk = nc.dram_tensor("Wk", [P, ND, H], FP8, kind="ExternalInput").ap()
    wv = nc.dram_tensor("Wv", [P, ND, H], FP16, kind="ExternalInput").ap()
    out = nc.dram_tensor("out", [BPC, S, H], FP32, kind="ExternalOutput").ap()

    import contextlib

    with contextlib.ExitStack() as ctx:
        consts = ctx.enter_context(tc.tile_pool(name="consts", bufs=1))
        big = ctx.enter_context(tc.tile_pool(name="big", bufs=1))
        vpool = ctx.enter_context(tc.tile_pool(name="vpool", bufs=2))
        fpool = ctx.enter_context(tc.tile_pool(name="fpool", bufs=2))
        stage = ctx.enter_context(tc.tile_pool(name="stage", bufs=4))
        small = ctx.enter_context(tc.tile_pool(name="small", bufs=4))
        dpool = ctx.enter_context(tc.tile_pool(name="dpool", bufs=2))
        outp = ctx.enter_context(tc.tile_pool(name="outp", bufs=6))
        psp = ctx.enter_context(tc.tile_pool(name="psp", bufs=4, space="PSUM"))

        ident32 = consts.tile([P, P], FP32)
        make_identity(nc, ident32)
        ident16 = consts.tile([P, P], FP16)
        nc.vector.tensor_copy(ident16, ident32)
        ones32 = consts.tile([1, 1], FP32)
        nc.vector.memset(ones32, 1.0)
        ones32c = consts.tile([P, 1], FP32)
        nc.vector.memset(ones32c, 1.0)
        ones16c = consts.tile([P, 1], FP16)
        nc.vector.memset(ones16c, 1.0)
        ones16r = consts.tile([1, GW], FP16)
        nc.vector.memset(ones16r, 1.0)

        # Weights arrive pre-chunked/pre-scaled; straight DMA.
        # wk first on the Sync ring (the opening kT-direct chains need only
        # wk + x8-q0); wq/wv go out on the GpSimd ring in parallel (first
        # needed by the projections, several microseconds in)
        wk_f = consts.tile([P, ND, H], FP8, name="wk_sb")
        nc.sync.dma_start(out=wk_f, in_=wk)
        wq_f = consts.tile([P, ND, H], FP8, name="wq_sb")
        nc.sync.dma_start(out=wq_f, in_=wq)
        wv_f = consts.tile([P, ND, H], FP16, name="wv_sb")

        def alloc_and_dma_x(b):
            # ---- phase 1: xT arrives pre-transposed — straight DMA ----
            xT8 = big.tile([P, ND, S], FP8, tag="xT8", bufs=2, name=f"xT8_{b}")
            xT16 = big.tile([P, ND, S], FP16, tag="xT16", bufs=2,
                            name=f"xT16_{b}")
            xsrc8 = x8[b].rearrange("(c p) s -> p c s", p=P)
            xsrc16 = x16[b].rearrange("(c p) s -> p c s", p=P)
            # x8 lands first (kT-direct + q/k projections need it); x16
            # (v path) follows
            for q4 in range(4):
                cols = slice(q4 * GW, (q4 + 1) * GW)
                nc.sync.dma_start(out=xT8[:, :, cols], in_=xsrc8[:, :, cols])
                if q4 == 0:
                    nc.sync.dma_start(out=xT16[:, :, cols],
                                      in_=xsrc16[:, :, cols])
            for q4 in range(1, 4):
                cols = slice(q4 * GW, (q4 + 1) * GW)
                nc.sync.dma_start(out=xT16[:, :, cols], in_=xsrc16[:, :, cols])
            kT = big.tile([P, NH, S], FP8, tag="kT", bufs=2, name=f"kT_{b}")
            return {"xT8": xT8, "xT16": xT16, "kT": kT}

        def emit_ktd_chain(tiles, jq, hc):
            # kT computed directly: kT[h, j] = sum_d Wk[d, h] xT[d, j]
            jcols = slice(jq * GW, (jq + 1) * GW)
            kt_ps = psp.tile([P, GW], FP32, tag="ps", name="kt_ps")
            for c in range(ND // 2):
                nc.tensor.matmul(
                    kt_ps,
                    wk_f[:, 2 * c:2 * c + 2, hc * P:(hc + 1) * P],
                    tiles["xT8"][:, 2 * c:2 * c + 2, jcols],
                    start=(c == 0), stop=(c == ND // 2 - 1),
                    perf_mode=DR)
            nc.vector.tensor_copy(out=tiles["kT"][:, hc, jcols], in_=kt_ps)

        cur = None
        for b in range(BPC):
            if cur is None:
                cur = alloc_and_dma_x(0)
                nc.sync.dma_start(out=wv_f, in_=wv)
                for jq in range(4):
                    for hc in range(NH):
                        emit_ktd_chain(cur, jq, hc)
            xT8, xT16, kT = cur["xT8"], cur["xT16"], cur["kT"]

            # ---- phase 2: projections, softmax pieces, transposed p ----
            pT = big.tile([P, NH, S], FP8, tag="pT", bufs=2, name=f"pT_{b}")
            # v rows scaled by 256/sk: fp8 copy for the f-GEMM, fp16
            # transients for the exact colsum
            v8_all = vpool.tile([P, NB, H], FP8, tag="v8", name=f"v8_{b}")
            rsk16_all = small.tile([P, NB], FP16, tag="rsk16", bufs=2)
            r8_all = small.tile([P, NB, 16], FP8, tag="r8", bufs=2)
            cs_ps = psp.tile([1, H], FP32, tag="ps2", bufs=2, name="cs_ps")
            eq_tiles = {}
            diag_tiles = {}

            def emit_tr(jb):
                tp = psp.tile([P, H], FP32, tag="ps", name="tpp")
                for hc in range(NH):
                    nc.tensor.matmul(
                        tp[:, hc * P:(hc + 1) * P],
                        eq_tiles[jb][:, hc * P:(hc + 1) * P], diag_tiles[jb],
                        start=True, stop=True)
                nc.vector.tensor_copy(
                    out=pT[:, :, jb * P:(jb + 1) * P],
                    in_=tp.rearrange("p (c f) -> p c f", c=NH))

            for ib in range(NB):
                q_ps = psp.tile([P, H], FP32, tag="ps", name="q_ps")
                k_ps = psp.tile([P, H], FP32, tag="ps", name="k_ps")
                for ps, wt in ((q_ps, wq_f), (k_ps, wk_f)):
                    for c in range(ND // 2):
                        nc.tensor.matmul(
                            ps, xT8[:, 2 * c:2 * c + 2, ib * P:(ib + 1) * P],
                            wt[:, 2 * c:2 * c + 2, :],
                            start=(c == 0), stop=(c == ND // 2 - 1),
                            perf_mode=DR)
                v_ps = psp.tile([P, H], FP32, tag="ps", name="v_ps")
                for dc in range(ND):
                    nc.tensor.matmul(
                        v_ps, xT16[:, dc, ib * P:(ib + 1) * P], wv_f[:, dc, :],
                        start=(dc == 0), stop=(dc == ND - 1))

                eq_sb = stage.tile([P, H], FP16, tag="eq", bufs=5)
                sq = small.tile([P, 1], FP32, tag="sq")
                nc.scalar.activation(eq_sb, q_ps, EXP, scale=1.0 / WS,
                                     accum_out=sq)
                rq = small.tile([P, 1], FP32, tag="rq")
                nc.vector.reciprocal(rq, sq)
                # diag(PS/sq): folds p-normalization and the fp8 pre-scale
                # into the p transpose matmul
                diag = stage.tile([P, P], FP16, tag="diag", bufs=5)
                nc.gpsimd.tensor_scalar(diag, ident16, rq, PS, op0=MUL,
                                        op1=MUL)

                ek_sb = stage.tile([P, H], FP16, tag="ek", bufs=2)
                sk = small.tile([P, 1], FP32, tag="sk")
                nc.scalar.activation(ek_sb, k_ps, EXP, scale=1.0 / WS,
                                     accum_out=sk)
                rsk = small.tile([P, 1], FP32, tag="rsk")
                nc.vector.reciprocal(rsk, sk)
                nc.vector.tensor_scalar_mul(
                    rsk16_all[:, ib:ib + 1], rsk, SK_SCALE)
                nc.vector.tensor_scalar_mul(
                    r8_all[:, ib, 0:1], rsk, SK_SCALE)
                # v' = v * rsk * SK_SCALE: fp16 transient for the exact
                # colsum, fp8 copy (via GpSimd) for the f-GEMM
                v16_sb = stage.tile([P, H], FP16, tag="v16", bufs=3)
                nc.vector.tensor_scalar(
                    v16_sb, v_ps, rsk, SK_SCALE, op0=MUL, op1=MUL)
                nc.gpsimd.tensor_copy(out=v8_all[:, ib, :], in_=v16_sb)
                nc.tensor.matmul(cs_ps, ones16c, v16_sb,
                                 start=(ib == 0), stop=(ib == NB - 1))

                eq_tiles[ib] = eq_sb
                diag_tiles[ib] = diag
                if ib >= 3:
                    emit_tr(ib - 3)

            # exact colsum(r'): free-dim reduce on DVE, then one tiny matmul
            # colsum finalization is deferred into the scores stream (see
            # below) so the phase-2-end Vector backlog never stalls the PE

            # ---- phase 3: scores (transposed, DR), f = e - 1, output ----
            def emit_scores(gp, jbs, fb):
                for jb in jbs:
                    s_ps = psp.tile([P, 2, GW], FP32, tag="ps2", bufs=2,
                                    name="s_ps")
                    for pair in range(2):
                        lhs = kT[:, 2 * pair:2 * pair + 2,
                                 jb * P:(jb + 1) * P]
                        for lg in range(2):
                            ig = 2 * gp + lg
                            nc.tensor.matmul(
                                s_ps[:, lg, :], lhs,
                                pT[:, 2 * pair:2 * pair + 2,
                                   ig * GW:(ig + 1) * GW],
                                start=(pair == 0), stop=(pair == 1),
                                perf_mode=DR)
                    e16 = stage.tile([P, 2 * GW], FP16, tag="e16", bufs=6)
                    nc.scalar.activation(
                        e16, s_ps.rearrange("p a f -> p (a f)"), EXP,
                        scale=1.0 / (PS * WS))
                    nc.vector.tensor_scalar(
                        fb[:, jb, :], e16, 1.0, None, op0=SUB)

            def emit_den(gp, fb):
                # denominators: colsum(r') broadcast + reversed DR matmuls
                # over the f residual, then the [1, 512] PSUM row is
                # transposed back onto partitions with 4 tiny fp32 matmuls.
                # both den chains first, then the rl transposes — the
                # PSUM->SBUF copy of den(lg0) hides under den(lg1)'s chain
                den_sbs = {}
                for lg in range(2):
                    den_ps = psp.tile([1, GW], FP32, tag="ps", name="den_ps")
                    nc.tensor.matmul(den_ps, csr_sb, ones16r,
                                     start=True, stop=False)
                    for t in range(NPAIR):
                        nc.tensor.matmul(
                            den_ps, r8_all[:, 2 * t:2 * t + 2, 0:1],
                            fb[:, 2 * t:2 * t + 2, lg * GW:(lg + 1) * GW],
                            start=False, stop=(t == NPAIR - 1),
                            perf_mode=DR)
                    den_sb = dpool.tile([1, GW], FP32, tag="den")
                    nc.vector.tensor_copy(den_sb, den_ps)
                    den_sbs[lg] = den_sb
                rl_sbs = {}
                for lg in range(2):
                    rl_ps = psp.tile([P, NG], FP32, tag="ps", name="rl_ps")
                    for il in range(NG):
                        nc.tensor.matmul(
                            rl_ps[:, il:il + 1],
                            den_sbs[lg][0:1, il * P:(il + 1) * P], ones32,
                            start=True, stop=True)
                    rl_sb = small.tile([P, NG], FP32, tag="rl", bufs=4)
                    nc.vector.reciprocal(rl_sb, rl_ps)
                    rl_sbs[lg] = rl_sb
                return rl_sbs

            def emit_out_chain(st, lg, il):
                ig = 2 * st["gp"] + lg
                ib = ig * NG + il
                o_ps = psp.tile([P, H], FP32, tag="ps", name="o_ps")
                nc.tensor.matmul(o_ps, ones16r[0:1, 0:P], st["cs"],
                                 start=True, stop=False)
                for t in range(NPAIR):
                    nc.tensor.matmul(
                        o_ps,
                        st["fb"][:, 2 * t:2 * t + 2,
                                 lg * GW + il * P:lg * GW + (il + 1) * P],
                        st["v8"][:, 2 * t:2 * t + 2, :],
                        start=False, stop=(t == NPAIR - 1),
                        perf_mode=DR)
                o_sb = outp.tile([P, H], FP32, tag="o")
                nc.vector.tensor_scalar_mul(
                    o_sb, o_ps, st["rl"][lg][:, il:il + 1])
                # output stores go out on the GpSimd queue so the next
                # batch's input loads (Sync queue) never queue behind them
                nc.gpsimd.dma_start(
                    out=out[st["b"], ib * P:(ib + 1) * P, :], in_=o_sb)

            # The scores loops are Scalar-bound (the PE needs ~0.9us per
            # row block but the exp takes ~1.2us), so PE-pure work is
            # interleaved into them to fill the bubbles:
            #   group 0: the NEXT batch's kT-direct chains (its x DMAs are
            #            prefetched here, overlapping this batch's compute)
            #            plus the PREVIOUS batch's deferred group-1 output
            #   group 1: group-0's output chains
            # The first 13 gp0 blocks are hoisted in front of the tail p
            # transposes so the latter never stall the PE on the exp
            # backlog; den(0) trails two gp1 blocks to hide the wait on
            # gp0's last f-subtract.
            nxt = alloc_and_dma_x(b + 1) if b + 1 < BPC else None
            ktd_todo = [(jq, hc) for jq in range(4) for hc in range(NH)] \
                if nxt is not None else []
            fb0 = fpool.tile([P, NB, 2 * GW], FP8, tag="f", name=f"f0_{b}")
            for jb in range(NB - 3):
                emit_scores(0, [jb], fb0)
                if ktd_todo:
                    emit_ktd_chain(nxt, *ktd_todo.pop(0))
                if jb == 0:
                    # deferred colsum(r') finalization: free-dim reduce on
                    # DVE, then one tiny matmul for the cross-partition sum
                    rsum = small.tile([P, 1], FP32, tag="rsum", bufs=2)
                    nc.vector.tensor_reduce(rsum, rsk16_all,
                                            mybir.AxisListType.X,
                                            mybir.AluOpType.add)
                    csr_ps = psp.tile([1, 1], FP32, tag="ps", name="csr_ps")
                    nc.tensor.matmul(csr_ps, rsum, ones32c,
                                     start=True, stop=True)
                    csr_sb = small.tile([1, 1], FP16, tag="csr", bufs=2)
                    nc.vector.tensor_copy(csr_sb, csr_ps)
                elif jb == 1:
                    cs_sb = dpool.tile([1, H], FP16, tag="cs")
                    nc.vector.tensor_copy(cs_sb, cs_ps)
            emit_tr(NB - 3)
            emit_tr(NB - 2)
            emit_tr(NB - 1)
            for jb in range(NB - 3, NB):
                emit_scores(0, [jb], fb0)
                if ktd_todo:
                    emit_ktd_chain(nxt, *ktd_todo.pop(0))
            fb1 = fpool.tile([P, NB, 2 * GW], FP8, tag="f", name=f"f1_{b}")
            emit_scores(1, [0], fb1)
            while ktd_todo:
                emit_ktd_chain(nxt, *ktd_todo.pop(0))
            emit_scores(1, [1], fb1)
            emit_scores(1, [2], fb1)
            rl0 = emit_den(0, fb0)
            st0 = {"b": b, "gp": 0, "fb": fb0, "v8": v8_all, "rl": rl0,
                   "cs": cs_sb}
            for jb in range(3, NB):
                emit_scores(1, [jb], fb1)
                if jb % 2 == 1:
                    c = jb // 2
                    emit_out_chain(st0, c // NG, c % NG)
            emit_out_chain(st0, 0, 0)
            rl1 = emit_den(1, fb1)
            st1 = {"b": b, "gp": 1, "fb": fb1, "v8": v8_all, "rl": rl1,
                   "cs": cs_sb}
            for lg in range(2):
                for il in range(NG):
                    emit_out_chain(st1, lg, il)
            cur = nxt


_NC_CACHE = {}


def _get_nc():
    if "nc" not in _NC_CACHE:
        nc = bacc.Bacc("TRN2", target_bir_lowering=False, debug=False)
        with tile.TileContext(nc) as tc:
            _emit(tc)
        nc.compile()
        _NC_CACHE["nc"] = nc
    return _NC_CACHE["nc"]


_NP8 = mybir.dt.np(FP8)


def _prep_w(w, scale, npdt):
    # [D, H] fp32 -> [128, ND, H] chunked on the contraction dim, *scale
    w = np.asarray(w, dtype=np.float32).reshape(ND, P, H).transpose(1, 0, 2)
    return np.ascontiguousarray(w * scale).astype(npdt)


def _run(inputs, trace=False, trace_cores=None):
    nc = _get_nc()
    x = np.asarray(inputs["x"], dtype=np.float32)
    wq = _prep_w(inputs["Wq"], WS, _NP8)
    wk = _prep_w(inputs["Wk"], WS, _NP8)
    wv = _prep_w(inputs["Wv"], 1.0, np.float16)
    in_maps = []
    for c in range(NCORES):
        xs = x[c * BPC:(c + 1) * BPC]
        xt = np.ascontiguousarray(xs.transpose(0, 2, 1))
        in_maps.append({
            "x8": xt.astype(_NP8), "x16": xt.astype(np.float16),
            "Wq": wq, "Wk": wk, "Wv": wv,
        })
    res = run_bass_kernel_spmd(
        nc, in_maps, core_ids=list(range(NCORES)),
        trace=trace, trace_cores=trace_cores)
    full = np.concatenate([res.results[c]["out"] for c in range(NCORES)], axis=0)
    return full, res


def kernel(**inputs) -> np.ndarray:
    out, _ = _run(inputs)
    return out
